# revision 59
# baseline (speedup 1.0000x reference)
"""Distributed causal attention kernel for 8 TRN2 NeuronCores.

Sharding: core c -> (batch b = c//2, head-group g = c%2).  Each core
computes attention for its batch over 8 of the 16 heads plus the partial
output projection (row-parallel Wo); the host sums the two partials per
batch and transposes back.

Device layout (per core):
  inputs  xq/xk/xv : x.T            [1024, 2048] bf16
          wq/wk/wv : W_g.T          [1024, 512]  bf16  (SCALE folded into wq)
          wo       : Wo[:,g-cols].T [512, 1024]  bf16
          tri      : [128,128] lower-step mask  tri[p,f] = (f >= p)
  output  out      : partial O.T    [1024, 2048] bf16 (host sums in f32)

Pipeline: qT/kT = Wg @ x.T (transposed), v natural [seq, 512];
S.T[sj,si] = k q.T per head (K=64, both heads CONCURRENT via PE row
tiling -- partitions 0-63 / 64-127 -> row_grp h0/h64, two PSUM banks);
P = exp(S.T) on ScalarE (logits are tiny -> no max subtraction);
causal mask = matmul N-range restriction + tri mask on diagonal blocks;
PV with ones-augmented v (M=65) -> unnormalised A.T + denominator row;
denominators reciprocal'd once per pair on DVE (full-65-partition op:
custom DVE needs base partition 0), broadcast via tiny K=1 col-tiled
matmuls; O.T = WoT.T @ A.T, last stripe staged into dead at[] columns
so the tail ships with 2 DMA triggers instead of 8.

Start: memset-fed HAM warmup (no DMA dependency) + few big input DMAs
(each dma_start costs ~680ns serialized sync-engine trigger time).
"""

import os

import numpy as np

import concourse.bass as bass
import concourse.tile as tile
from concourse import bacc, mybir
from concourse.bass import MemorySpace

F32 = mybir.dt.float32
BF16 = mybir.dt.bfloat16
AF = mybir.ActivationFunctionType

B, S, DIM, H = 4, 2048, 1024, 16
HD = DIM // H          # 64
SCALE = HD ** -0.5
NCORES = 8
DG = DIM // 2          # 512 head dims per core (8 heads)
NPAIR = 4              # head pairs per core
SI = 512               # si chunk (query positions per attention tile)
NSI = S // SI          # 4
SJ = 128               # sj chunk (key positions per matmul)
AC = 512               # phase-A seq chunk
NAC = S // AC          # 4
KC = DIM // 128        # 8 contraction chunks for projections

LAST_RESULTS = None


def _build_core_kernel():
    nc = bacc.Bacc(
        "TRN2", target_bir_lowering=False, debug=False, num_devices=NCORES
    )

    xq = nc.dram_tensor("xq", [DIM, S], BF16, kind="ExternalInput").ap()
    xk = nc.dram_tensor("xk", [DIM, S], BF16, kind="ExternalInput").ap()
    xv = nc.dram_tensor("xv", [DIM, S], BF16, kind="ExternalInput").ap()
    wq = nc.dram_tensor("wq", [DIM, DG], BF16, kind="ExternalInput").ap()
    wk = nc.dram_tensor("wk", [DIM, DG], BF16, kind="ExternalInput").ap()
    wv = nc.dram_tensor("wv", [DIM, DG], BF16, kind="ExternalInput").ap()
    wo = nc.dram_tensor("wo", [DG, DIM], BF16, kind="ExternalInput").ap()
    tri = nc.dram_tensor("tri", [128, 128], BF16, kind="ExternalInput").ap()
    out = nc.dram_tensor("out", [DIM, S], BF16, kind="ExternalOutput").ap()

    # partition-tiled DRAM views
    xq_v = xq.rearrange("(kc p) s -> p kc s", p=128)   # [128, 8, 2048]
    xk_v = xk.rearrange("(kc p) s -> p kc s", p=128)
    xv_v = xv.rearrange("(kc p) s -> p kc s", p=128)
    wq_v = wq.rearrange("(kc p) m -> p kc m", p=128)   # [128, 8, 512]
    wk_v = wk.rearrange("(kc p) m -> p kc m", p=128)
    wv_v = wv.rearrange("(kc p) m -> p kc m", p=128)
    wo_v = wo.rearrange("(kt p) m -> p kt m", p=128)   # [128, 4, 1024]
    out_v = out.rearrange("(mt p) s -> p mt s", p=128)  # [128, 8, 2048]

    with tile.TileContext(nc) as tc:
        with (
            tc.tile_pool(name="persist", bufs=1) as persist,
            tc.tile_pool(name="cw", bufs=1) as cwpool,
            tc.tile_pool(name="co", bufs=4) as copool,
            tc.tile_pool(name="bcp", bufs=1) as bcpool,
        ):
            # persistent SBUF tensors
            qT = persist.tile([128, NPAIR, S], BF16)        # [64l+d, pair, si]
            kT = persist.tile([128, NPAIR, S], BF16)
            vaug = persist.tile([128, S // SJ, 8, HD + 1], BF16)  # [sj, j, h, d|1]
            at = persist.tile([128, NPAIR, S], BF16)        # unnorm A.T
            rden = persist.tile([65, NPAIR, 2, S], BF16)  # 1/den rows @ p64
            ones64 = persist.tile([65, 64], BF16)
            tri_sb = persist.tile([128, 128], BF16)
            warm_sb = persist.tile([128, 128], BF16)
            wo_bf = cwpool.tile([128, 4, DIM], BF16, tag="wo16")

            # ones column of vaug
            nc.vector.memset(vaug[:, :, :, HD], 1.0)
            nc.vector.memset(warm_sb[:], 0.125)
            nc.vector.memset(ones64[:], 1.0)

            def norm_unit(si0, siw, p, bpsum):
                ssl = slice(si0, si0 + siw)
                # broadcast both heads' 1/den rows (bf16, from end_pair)
                # via two tiny col-tiled K=1 matmuls, then one multiply
                bc = bpsum.tile([128, siw], F32, tag="bc", name="bc")
                for l in range(2):
                    nc.tensor.matmul(
                        bc[64 * l:64 * l + 64, :],
                        ones64[64:65, 0:64],
                        rden[64:65, p, l, ssl],
                        start=True,
                        stop=True,
                    )
                nc.vector.tensor_mul(at[:, p, ssl], at[:, p, ssl], bc[:, :])

            def wo_chain(si0, siw, mt, cpsum, stage=False):
                ssl = slice(si0, si0 + siw)
                ps = cpsum.tile([128, siw], F32, tag="cps", name="cps")
                for kt in range(4):
                    nc.tensor.matmul(
                        ps[:, :],
                        wo_bf[:, kt, mt * 128:(mt + 1) * 128],
                        at[:, kt, ssl],
                        start=(kt == 0),
                        stop=(kt == 3),
                    )
                if not stage:
                    osb = copool.tile([128, siw], BF16, tag="osb", name="osb")
                    nc.vector.tensor_copy(osb[:, :], ps[:, :])
                    nc.sync.dma_start(out=out_v[:, mt, ssl], in_=osb[:, :])
                    return
                # final stripe: stage into dead at[] columns (stripes 0-1
                # fully consumed) and ship 4 tiles per dma_start -- the
                # per-tile DMA triggers (~680ns serialized on the sync
                # engine) were most of the kernel tail.
                nc.vector.tensor_copy(
                    at[:, mt % 4, (mt // 4) * siw:(mt // 4 + 1) * siw],
                    ps[:, :],
                )
                if mt % 4 == 3:
                    h = mt // 4
                    nc.sync.dma_start(
                        out=out_v[:, 4 * h:4 * h + 4, ssl],
                        in_=at[:, :, h * siw:(h + 1) * siw],
                    )

            # Emission is interleaved so every engine's serial stream stays
            # busy: attention units (QK->exp->PV, ScalarE-bound) are the
            # backbone; projection chains / Wo chains (PE-bound) are woven
            # between them as fillers.
            with (
                tc.tile_pool(name="pt", bufs=8) as ptpool,
            ):

                def make_stripe(si0, siw, stp, opp, pair_done=None):
                    """Emission units for attention rows [si0, si0+siw)."""
                    ssl = slice(si0, si0 + siw)
                    njs = (si0 + siw) // SJ
                    units = []
                    for p in range(NPAIR):
                        state = {}

                        def start_pair(p=p, state=state):
                            state["o2"] = opp.tile(
                                [65, 2, siw], F32, tag="o2", name="o2"
                            )

                        def unit(j, p=p, state=state):
                            sj0 = j * SJ
                            d0 = sj0 - si0
                            r0 = max(0, d0)
                            # st always spans 2 PSUM banks: the two QK
                            # matmuls are concurrent row tiles, and row
                            # tiles must not write the same bank.
                            st2 = stp.tile([128, 2, SI], F32, tag="st", name="st")
                            pt = ptpool.tile([128, 2, siw], BF16, tag="pt", name="pt")
                            for l in range(2):
                                lsl = slice(64 * l, 64 * l + 64)
                                nc.tensor.matmul(
                                    st2[:, l, r0:siw],
                                    kT[lsl, p, sj0:sj0 + SJ],
                                    qT[lsl, p, si0 + r0:si0 + siw],
                                    start=True,
                                    stop=True,
                                )
                            nc.scalar.activation(
                                pt[:, :, r0:siw], st2[:, :, r0:siw], AF.Exp
                            )
                            if d0 >= 0:
                                for l in range(2):
                                    nc.vector.tensor_mul(
                                        pt[:, l, d0:d0 + 128],
                                        pt[:, l, d0:d0 + 128],
                                        tri_sb[:, :],
                                    )
                            for l in range(2):
                                nc.tensor.matmul(
                                    state["o2"][:, l, r0:siw],
                                    vaug[:, j, 2 * p + l, :],
                                    pt[:, l, r0:siw],
                                    start=(j == 0),
                                    stop=(j == njs - 1),
                                )

                        def end_pair(p=p, state=state, ssl=ssl):
                            o2 = state["o2"]
                            for l in range(2):
                                nc.vector.tensor_copy(
                                    at[64 * l:64 * l + 64, p, ssl], o2[0:HD, l, :]
                                )
                            # reciprocal of both heads' denominator rows;
                            # custom-DVE ops need base partition 0, so run
                            # over all 65 partitions -- rows 0-63 get junk
                            # reciprocals of A values that are never read.
                            # Then a lane-aligned cast of the one good row
                            # into the persistent bf16 rden.
                            rscr = bcpool.tile(
                                [65, 2, siw], F32, tag="rscr", name="rscr"
                            )
                            nc.vector.reciprocal_approx_fast(
                                rscr[0:65, :, :], o2[0:65, :, :]
                            )
                            nc.vector.tensor_copy(
                                rden[64:65, p, :, ssl], rscr[64:65, :, :]
                            )

                        units.append(start_pair)
                        for j in range(njs):
                            units.append(lambda j=j, u=unit: u(j))
                        units.append(end_pair)
                        if pair_done is not None:
                            units.append(lambda p=p: pair_done(p))
                    return units

                def emit_interleaved(units, fillers, tail_frac=1.0):
                    """Emit units with fillers distributed evenly between.

                    tail_frac < 1 exhausts the fillers by that fraction of
                    the unit stream, leaving the last units filler-free so
                    their completion chain gets idle engines.
                    """
                    U, F = len(units), len(fillers)
                    eff = max(1, int(U * tail_frac))
                    fi = 0
                    for k, u in enumerate(units):
                        u()
                        want = min(F, (k + 1) * F // eff)
                        while fi < want:
                            fillers[fi]()
                            fi += 1
                    while fi < F:
                        fillers[fi]()
                        fi += 1

                with (
                    tc.tile_pool(name="ax", bufs=3) as xpool,
                    tc.tile_pool(name="aw", bufs=1) as wpool,
                    tc.tile_pool(name="aps", bufs=2, space=MemorySpace.PSUM) as apsum,
                    tc.tile_pool(name="stps", bufs=2, space=MemorySpace.PSUM) as stps,
                    tc.tile_pool(name="ops", bufs=1, space=MemorySpace.PSUM) as ops,
                ):
                    wq_sb = wpool.tile([128, KC, DG], BF16, tag="wq")
                    wk_sb = wpool.tile([128, KC, DG], BF16, tag="wk")
                    wv_sb = wpool.tile([128, KC, DG], BF16, tag="wv")
                    w_sb = {"q": wq_sb, "k": wk_sb, "v": wv_sb}
                    x_view = {"q": xq_v, "k": xk_v, "v": xv_v}
                    x_tiles = {}

                    def dma_x(t, n):
                        xt = xpool.tile(
                            [128, KC, AC], BF16, tag="x", name=f"x_{t}{n}"
                        )
                        nc.sync.dma_start(
                            out=xt[:], in_=x_view[t][:, :, n * AC:(n + 1) * AC]
                        )
                        x_tiles[(t, n)] = xt

                    def chain_qk(t, n, p):
                        sl = slice(n * AC, (n + 1) * AC)
                        xt = x_tiles[(t, n)]
                        ps = apsum.tile([128, AC], F32, tag="aps", name="aps")
                        for kc in range(KC):
                            nc.tensor.matmul(
                                ps[:, :],
                                w_sb[t][:, kc, p * 128:(p + 1) * 128],
                                xt[:, kc, :],
                                start=(kc == 0),
                                stop=(kc == KC - 1),
                            )
                        nc.vector.tensor_copy(
                            (qT if t == "q" else kT)[:, p, sl], ps[:, :]
                        )

                    def chain_v(n, mm):
                        xt = x_tiles[("v", n)]
                        j = n * (AC // 128) + mm
                        ps = apsum.tile([128, DG], F32, tag="aps", name="apsv")
                        for kc in range(KC):
                            nc.tensor.matmul(
                                ps[:, :],
                                xt[:, kc, mm * 128:(mm + 1) * 128],
                                w_sb["v"][:, kc, :],
                                start=(kc == 0),
                                stop=(kc == KC - 1),
                            )
                        nc.vector.tensor_copy(vaug[:, j, :, 0:HD], ps[:, :])

                    def chunk_fillers(n):
                        fs = [lambda t=t, n=n: dma_x(t, n) for t in ("q", "k", "v")]
                        for p in range(NPAIR):
                            fs.append(lambda p=p, n=n: chain_qk("q", n, p))
                        for p in range(NPAIR):
                            fs.append(lambda p=p, n=n: chain_qk("k", n, p))
                        for mm in range(AC // 128):
                            fs.append(lambda mm=mm, n=n: chain_v(n, mm))
                        return fs

                    # chunk 0: each dma_start costs ~680ns of serialized
                    # trigger time on the sync engine, so use few, large
                    # DMAs: two halves each for wq and xq0 (kc 0-3 / 4-7)
                    # keep the first chain steps consumable early.
                    xt0 = xpool.tile([128, KC, AC], BF16, tag="x", name="x_q0")
                    x_tiles[("q", 0)] = xt0
                    for h in range(2):
                        ksl = slice(4 * h, 4 * h + 4)
                        nc.sync.dma_start(
                            out=wq_sb[:, ksl, :], in_=wq_v[:, ksl, :]
                        )
                        nc.sync.dma_start(
                            out=xt0[:, ksl, :], in_=xq_v[:, ksl, 0:AC]
                        )
                    # HAM warmup: memset-fed throwaway matmuls (no DMA
                    # dependency, so they start ~immediately) woven between
                    # the DMA-gated kc steps of the first q chain -- keeps
                    # the PE busy from t=0 so the SHORT window trips early
                    # and real matmuls run at 2.4 GHz.  Borrows the ops
                    # (o2) PSUM bank, which is dead until stripe-0 PV.
                    warm = ops.tile([64, 128], F32, tag="o2", name="warm")

                    def warm_mms(n):
                        # alternate stationary slices so LDWEIGHTS of the
                        # next warm MM ping-pongs into the background
                        # weight buffer instead of serializing
                        for k in range(n):
                            c0 = 64 * (k % 2)
                            nc.tensor.matmul(
                                warm[:, :], warm_sb[:, c0:c0 + 64],
                                warm_sb[:, :],
                                start=True, stop=True,
                            )

                    # all warm MMs upfront: the PE is in-order, so warm MMs
                    # emitted after a DMA-gated matmul would stall with it.
                    warm_mms(26)
                    # kc-major: 4 pair-chains advance together, one psum each?
                    # psum only has 2 aps slots here, so do pairs in twos.
                    for ph in range(2):
                        pss = [
                            apsum.tile([128, AC], F32, tag="aps", name="q0ps")
                            for _ in range(2)
                        ]
                        for kc in range(KC):
                            for pi in range(2):
                                p = 2 * ph + pi
                                nc.tensor.matmul(
                                    pss[pi][:, :],
                                    wq_sb[:, kc, p * 128:(p + 1) * 128],
                                    xt0[:, kc, :],
                                    start=(kc == 0),
                                    stop=(kc == KC - 1),
                                )
                        for pi in range(2):
                            nc.vector.tensor_copy(
                                qT[:, 2 * ph + pi, 0:AC], pss[pi][:, :]
                            )
                    dma_x("k", 0)
                    nc.sync.dma_start(out=wk_sb[:], in_=wk_v[:, :, :])
                    nc.sync.dma_start(out=tri_sb[:], in_=tri[:, :])
                    nc.sync.dma_start(out=wv_sb[:], in_=wv_v[:, :, :])
                    dma_x("v", 0)
                    for p in range(NPAIR):
                        chain_qk("k", 0, p)
                    for mm in range(AC // 128):
                        chain_v(0, mm)

                    emit_interleaved(
                        make_stripe(0, SI, stps, ops), chunk_fillers(1)
                    )
                    emit_interleaved(
                        make_stripe(SI, SI, stps, ops), chunk_fillers(2)
                    )
                    emit_interleaved(
                        make_stripe(2 * SI, SI, stps, ops), chunk_fillers(3)
                    )

                # ---- last stripe interleaves with normalise + Wo (1-buf
                # ---- psum pools; stalls absorb into exp waits)
                with (
                    tc.tile_pool(name="st2p", bufs=2, space=MemorySpace.PSUM) as stps2,
                    tc.tile_pool(name="ops2", bufs=1, space=MemorySpace.PSUM) as ops2,
                    tc.tile_pool(name="cps1", bufs=1, space=MemorySpace.PSUM) as cps1,
                    tc.tile_pool(name="bps1", bufs=1, space=MemorySpace.PSUM) as bps1,
                ):
                    # dependency-free warm MMs bridge the pool-transition
                    # stall here: if the PE idles >one HAM window at this
                    # boundary, the clock halves and takes ~17us to recover
                    wps = bps1.tile([64, 128], F32, tag="bc", name="wps")
                    for k in range(8):
                        c0 = 64 * (k % 2)
                        nc.tensor.matmul(
                            wps[:, :], warm_sb[:, c0:c0 + 64], warm_sb[:, :],
                            start=True, stop=True,
                        )
                    c_fillers = [
                        lambda: nc.sync.dma_start(out=wo_bf[:], in_=wo_v[:, :, :])
                    ]
                    for i in range(3):
                        for p in range(NPAIR):
                            c_fillers.append(
                                lambda i=i, p=p: norm_unit(i * SI, SI, p, bps1)
                            )
                        for mt in range(8):
                            c_fillers.append(
                                lambda i=i, mt=mt: wo_chain(i * SI, SI, mt, cps1)
                            )
                    emit_interleaved(
                        make_stripe(
                            3 * SI, SI, stps2, ops2,
                            pair_done=lambda p: norm_unit(3 * SI, SI, p, bps1),
                        ),
                        c_fillers,
                        tail_frac=0.9,
                    )

            # tail: final stripe Wo in kt-major waves of 4 chains -- the
            # 12 kt<3 matmuls of a wave have no dependency on the last
            # pair's normalisation, so they fill the norm(3,3) gate stall
            # that a per-chain loop would hit on its first kt=3.  Output
            # DMA groups taper (4,2,1,1 tiles) so the final transfer that
            # nothing can overlap is only 128KB.
            with (
                tc.tile_pool(name="cps2", bufs=4, space=MemorySpace.PSUM) as cps2,
            ):
                fsl = slice(3 * SI, 4 * SI)
                for w in range(2):
                    pss = [
                        cps2.tile([128, SI], F32, tag="cps", name="cps")
                        for _ in range(4)
                    ]
                    for kt in range(4):
                        for q in range(4):
                            mt = 4 * w + q
                            nc.tensor.matmul(
                                pss[q][:, :],
                                wo_bf[:, kt, mt * 128:(mt + 1) * 128],
                                at[:, kt, fsl],
                                start=(kt == 0),
                                stop=(kt == 3),
                            )
                    for q in range(4):
                        mt = 4 * w + q
                        nc.vector.tensor_copy(
                            at[:, q, w * SI:(w + 1) * SI], pss[q][:, :]
                        )
                        if w == 0 and q == 3:
                            nc.sync.dma_start(
                                out=out_v[:, 0:4, fsl],
                                in_=at[:, :, 0:SI],
                            )
                        elif w == 1 and q == 1:
                            nc.sync.dma_start(
                                out=out_v[:, 4:6, fsl],
                                in_=at[:, 0:2, SI:2 * SI],
                            )
                        elif w == 1 and q >= 2:
                            nc.sync.dma_start(
                                out=out_v[:, 4 + q:5 + q, fsl],
                                in_=at[:, q:q + 1, SI:2 * SI],
                            )

    nc.compile()
    return nc


_NC_CACHE = {}


def _get_nc():
    if "nc" not in _NC_CACHE:
        _NC_CACHE["nc"] = _build_core_kernel()
    return _NC_CACHE["nc"]


def make_in_maps(query, key, value, Wq, Wk, Wv, Wo):
    import ml_dtypes

    bf = ml_dtypes.bfloat16
    tri = (np.arange(128)[None, :] >= np.arange(128)[:, None]).astype(bf)
    # each batch's transposed activations are shared by its two cores;
    # build them once (the bf16 casts are the expensive part on host)
    xq_b = [np.ascontiguousarray(query[b].T).astype(bf) for b in range(B)]
    xk_b = [np.ascontiguousarray(key[b].T).astype(bf) for b in range(B)]
    xv_b = [np.ascontiguousarray(value[b].T).astype(bf) for b in range(B)]
    w_g = []
    for g in range(2):
        rows = slice(g * DG, (g + 1) * DG)
        w_g.append({
            "wq": np.ascontiguousarray((Wq[rows, :] * SCALE).T).astype(bf),
            "wk": np.ascontiguousarray(Wk[rows, :].T).astype(bf),
            "wv": np.ascontiguousarray(Wv[rows, :].T).astype(bf),
            "wo": np.ascontiguousarray(Wo[:, rows].T).astype(bf),
        })
    in_maps = []
    for c in range(NCORES):
        b, g = c // 2, c % 2
        in_maps.append({
            "xq": xq_b[b], "xk": xk_b[b], "xv": xv_b[b],
            **w_g[g], "tri": tri,
        })
    return in_maps


def kernel(query, key, value, attn_mask, Wq, Wk, Wv, Wo):
    global LAST_RESULTS
    from concourse.bass_utils import run_bass_kernel_spmd

    query = np.asarray(query, np.float32)
    key = np.asarray(key, np.float32)
    value = np.asarray(value, np.float32)
    Wq = np.asarray(Wq, np.float32)
    Wk = np.asarray(Wk, np.float32)
    Wv = np.asarray(Wv, np.float32)
    Wo = np.asarray(Wo, np.float32)

    nc = _get_nc()
    in_maps = make_in_maps(query, key, value, Wq, Wk, Wv, Wo)
    res = run_bass_kernel_spmd(
        nc,
        in_maps,
        core_ids=list(range(NCORES)),
        trace=bool(int(os.environ.get("KERNEL_TRACE", "0"))),
    )
    LAST_RESULTS = res

    full = np.empty((B, S, DIM), np.float32)
    for b in range(B):
        full[b] = (
            res.results[2 * b]["out"].astype(np.float32)
            + res.results[2 * b + 1]["out"].astype(np.float32)
        ).T
    return full



# revision 61
# speedup vs baseline: 1.0151x; 1.0151x over previous
"""Distributed causal attention kernel for 8 TRN2 NeuronCores.

Sharding: core c -> (batch b = c//2, head-group g = c%2).  Each core
computes attention for its batch over 8 of the 16 heads plus the partial
output projection (row-parallel Wo); the host sums the two partials per
batch and transposes back.

Device layout (per core):
  inputs  xq/xk/xv : x.T            [1024, 2048] bf16
          wq/wk/wv : W_g.T          [1024, 512]  bf16  (SCALE folded into wq)
          wo       : Wo[:,g-cols].T [512, 1024]  bf16
          tri      : [128,128] lower-step mask  tri[p,f] = (f >= p)
  output  out      : partial O.T    [1024, 2048] bf16 (host sums in f32)

Pipeline: qT/kT = Wg @ x.T (transposed), v natural [seq, 512];
S.T[sj,si] = k q.T per head (K=64, both heads CONCURRENT via PE row
tiling -- partitions 0-63 / 64-127 -> row_grp h0/h64, two PSUM banks);
P = exp(S.T) on ScalarE (logits are tiny -> no max subtraction);
causal mask = matmul N-range restriction + tri mask on diagonal blocks;
PV with ones-augmented v (M=65) -> unnormalised A.T + denominator row;
denominators reciprocal'd once per pair on DVE (full-65-partition op:
custom DVE needs base partition 0), broadcast via tiny K=1 col-tiled
matmuls; O.T = WoT.T @ A.T, last stripe staged into dead at[] columns
so the tail ships with 2 DMA triggers instead of 8.

Start: memset-fed HAM warmup (no DMA dependency) + few big input DMAs
(each dma_start costs ~680ns serialized sync-engine trigger time).
"""

import os

import numpy as np

import concourse.bass as bass
import concourse.tile as tile
from concourse import bacc, mybir
from concourse.bass import MemorySpace

F32 = mybir.dt.float32
BF16 = mybir.dt.bfloat16
AF = mybir.ActivationFunctionType

B, S, DIM, H = 4, 2048, 1024, 16
HD = DIM // H          # 64
SCALE = HD ** -0.5
NCORES = 8
DG = DIM // 2          # 512 head dims per core (8 heads)
NPAIR = 4              # head pairs per core
SI = 512               # si chunk (query positions per attention tile)
NSI = S // SI          # 4
SJ = 128               # sj chunk (key positions per matmul)
AC = 512               # phase-A seq chunk
NAC = S // AC          # 4
KC = DIM // 128        # 8 contraction chunks for projections

LAST_RESULTS = None


def _build_core_kernel():
    nc = bacc.Bacc(
        "TRN2", target_bir_lowering=False, debug=False, num_devices=NCORES
    )

    xq = nc.dram_tensor("xq", [DIM, S], BF16, kind="ExternalInput").ap()
    xk = nc.dram_tensor("xk", [DIM, S], BF16, kind="ExternalInput").ap()
    xv = nc.dram_tensor("xv", [DIM, S], BF16, kind="ExternalInput").ap()
    wq = nc.dram_tensor("wq", [DIM, DG], BF16, kind="ExternalInput").ap()
    wk = nc.dram_tensor("wk", [DIM, DG], BF16, kind="ExternalInput").ap()
    wv = nc.dram_tensor("wv", [DIM, DG], BF16, kind="ExternalInput").ap()
    wo = nc.dram_tensor("wo", [DG, DIM], BF16, kind="ExternalInput").ap()
    tri = nc.dram_tensor("tri", [128, 128], BF16, kind="ExternalInput").ap()
    out = nc.dram_tensor("out", [DIM, S], BF16, kind="ExternalOutput").ap()

    # partition-tiled DRAM views
    xq_v = xq.rearrange("(kc p) s -> p kc s", p=128)   # [128, 8, 2048]
    xk_v = xk.rearrange("(kc p) s -> p kc s", p=128)
    xv_v = xv.rearrange("(kc p) s -> p kc s", p=128)
    wq_v = wq.rearrange("(kc p) m -> p kc m", p=128)   # [128, 8, 512]
    wk_v = wk.rearrange("(kc p) m -> p kc m", p=128)
    wv_v = wv.rearrange("(kc p) m -> p kc m", p=128)
    wo_v = wo.rearrange("(kt p) m -> p kt m", p=128)   # [128, 4, 1024]
    out_v = out.rearrange("(mt p) s -> p mt s", p=128)  # [128, 8, 2048]

    with tile.TileContext(nc) as tc:
        with (
            tc.tile_pool(name="persist", bufs=1) as persist,
            tc.tile_pool(name="cw", bufs=1) as cwpool,
            tc.tile_pool(name="co", bufs=4) as copool,
            tc.tile_pool(name="bcp", bufs=1) as bcpool,
        ):
            # persistent SBUF tensors
            qT = persist.tile([128, NPAIR, S], BF16)        # [64l+d, pair, si]
            kT = persist.tile([128, NPAIR, S], BF16)
            vaug = persist.tile([128, S // SJ, 8, HD + 1], BF16)  # [sj, j, h, d|1]
            at = persist.tile([128, NPAIR, S], BF16)        # unnorm A.T
            rden = persist.tile([65, NPAIR, 2, S], BF16)  # 1/den rows @ p64
            ones64 = persist.tile([65, 64], BF16)
            tri_sb = persist.tile([128, 128], BF16)
            warm_sb = persist.tile([128, 128], BF16)
            wo_bf = cwpool.tile([128, 4, DIM], BF16, tag="wo16")

            # ones column of vaug
            nc.vector.memset(vaug[:, :, :, HD], 1.0)
            nc.vector.memset(warm_sb[:], 0.125)
            nc.vector.memset(ones64[:], 1.0)

            def norm_unit(si0, siw, p, bpsum):
                ssl = slice(si0, si0 + siw)
                # broadcast both heads' 1/den rows (bf16, from end_pair)
                # via two tiny col-tiled K=1 matmuls, then one multiply
                bc = bpsum.tile([128, siw], F32, tag="bc", name="bc")
                for l in range(2):
                    nc.tensor.matmul(
                        bc[64 * l:64 * l + 64, :],
                        ones64[64:65, 0:64],
                        rden[64:65, p, l, ssl],
                        start=True,
                        stop=True,
                    )
                nc.vector.tensor_mul(at[:, p, ssl], at[:, p, ssl], bc[:, :])

            def wo_chain(si0, siw, mt, cpsum, stage=False):
                ssl = slice(si0, si0 + siw)
                ps = cpsum.tile([128, siw], F32, tag="cps", name="cps")
                for kt in range(4):
                    nc.tensor.matmul(
                        ps[:, :],
                        wo_bf[:, kt, mt * 128:(mt + 1) * 128],
                        at[:, kt, ssl],
                        start=(kt == 0),
                        stop=(kt == 3),
                    )
                if not stage:
                    osb = copool.tile([128, siw], BF16, tag="osb", name="osb")
                    nc.vector.tensor_copy(osb[:, :], ps[:, :])
                    nc.sync.dma_start(out=out_v[:, mt, ssl], in_=osb[:, :])
                    return
                # final stripe: stage into dead at[] columns (stripes 0-1
                # fully consumed) and ship 4 tiles per dma_start -- the
                # per-tile DMA triggers (~680ns serialized on the sync
                # engine) were most of the kernel tail.
                nc.vector.tensor_copy(
                    at[:, mt % 4, (mt // 4) * siw:(mt // 4 + 1) * siw],
                    ps[:, :],
                )
                if mt % 4 == 3:
                    h = mt // 4
                    nc.sync.dma_start(
                        out=out_v[:, 4 * h:4 * h + 4, ssl],
                        in_=at[:, :, h * siw:(h + 1) * siw],
                    )

            # Emission is interleaved so every engine's serial stream stays
            # busy: attention units (QK->exp->PV, ScalarE-bound) are the
            # backbone; projection chains / Wo chains (PE-bound) are woven
            # between them as fillers.
            with (
                tc.tile_pool(name="pt", bufs=8) as ptpool,
            ):

                def make_stripe(si0, siw, stp, opp, pair_done=None):
                    """Emission units for attention rows [si0, si0+siw)."""
                    ssl = slice(si0, si0 + siw)
                    njs = (si0 + siw) // SJ
                    units = []
                    for p in range(NPAIR):
                        state = {}

                        def start_pair(p=p, state=state):
                            state["o2"] = opp.tile(
                                [65, 2, siw], F32, tag="o2", name="o2"
                            )

                        def unit(j, p=p, state=state):
                            sj0 = j * SJ
                            d0 = sj0 - si0
                            r0 = max(0, d0)
                            # st always spans 2 PSUM banks: the two QK
                            # matmuls are concurrent row tiles, and row
                            # tiles must not write the same bank.
                            st2 = stp.tile([128, 2, SI], F32, tag="st", name="st")
                            pt = ptpool.tile([128, 2, siw], BF16, tag="pt", name="pt")
                            for l in range(2):
                                lsl = slice(64 * l, 64 * l + 64)
                                nc.tensor.matmul(
                                    st2[:, l, r0:siw],
                                    kT[lsl, p, sj0:sj0 + SJ],
                                    qT[lsl, p, si0 + r0:si0 + siw],
                                    start=True,
                                    stop=True,
                                )
                            nc.scalar.activation(
                                pt[:, :, r0:siw], st2[:, :, r0:siw], AF.Exp
                            )
                            if d0 >= 0:
                                for l in range(2):
                                    nc.vector.tensor_mul(
                                        pt[:, l, d0:d0 + 128],
                                        pt[:, l, d0:d0 + 128],
                                        tri_sb[:, :],
                                    )
                            for l in range(2):
                                nc.tensor.matmul(
                                    state["o2"][:, l, r0:siw],
                                    vaug[:, j, 2 * p + l, :],
                                    pt[:, l, r0:siw],
                                    start=(j == 0),
                                    stop=(j == njs - 1),
                                )

                        def end_pair(p=p, state=state, ssl=ssl):
                            o2 = state["o2"]
                            for l in range(2):
                                nc.vector.tensor_copy(
                                    at[64 * l:64 * l + 64, p, ssl], o2[0:HD, l, :]
                                )
                            # reciprocal of both heads' denominator rows;
                            # custom-DVE ops need base partition 0, so run
                            # over all 65 partitions -- rows 0-63 get junk
                            # reciprocals of A values that are never read.
                            # Then a lane-aligned cast of the one good row
                            # into the persistent bf16 rden.
                            rscr = bcpool.tile(
                                [65, 2, siw], F32, tag="rscr", name="rscr"
                            )
                            nc.vector.reciprocal_approx_fast(
                                rscr[0:65, :, :], o2[0:65, :, :]
                            )
                            nc.vector.tensor_copy(
                                rden[64:65, p, :, ssl], rscr[64:65, :, :]
                            )

                        units.append(start_pair)
                        for j in range(njs):
                            units.append(lambda j=j, u=unit: u(j))
                        units.append(end_pair)
                        if pair_done is not None:
                            units.append(lambda p=p: pair_done(p))
                    return units

                def emit_interleaved(units, fillers, tail_frac=1.0):
                    """Emit units with fillers distributed evenly between.

                    tail_frac < 1 exhausts the fillers by that fraction of
                    the unit stream, leaving the last units filler-free so
                    their completion chain gets idle engines.
                    """
                    U, F = len(units), len(fillers)
                    eff = max(1, int(U * tail_frac))
                    fi = 0
                    for k, u in enumerate(units):
                        u()
                        want = min(F, (k + 1) * F // eff)
                        while fi < want:
                            fillers[fi]()
                            fi += 1
                    while fi < F:
                        fillers[fi]()
                        fi += 1

                with (
                    tc.tile_pool(name="ax", bufs=3) as xpool,
                    tc.tile_pool(name="aw", bufs=1) as wpool,
                    tc.tile_pool(name="aps", bufs=2, space=MemorySpace.PSUM) as apsum,
                    tc.tile_pool(name="stps", bufs=2, space=MemorySpace.PSUM) as stps,
                    tc.tile_pool(name="ops", bufs=1, space=MemorySpace.PSUM) as ops,
                ):
                    wq_sb = wpool.tile([128, KC, DG], BF16, tag="wq")
                    wk_sb = wpool.tile([128, KC, DG], BF16, tag="wk")
                    wv_sb = wpool.tile([128, KC, DG], BF16, tag="wv")
                    w_sb = {"q": wq_sb, "k": wk_sb, "v": wv_sb}
                    x_view = {"q": xq_v, "k": xk_v, "v": xv_v}
                    x_tiles = {}

                    def dma_x(t, n):
                        xt = xpool.tile(
                            [128, KC, AC], BF16, tag="x", name=f"x_{t}{n}"
                        )
                        nc.sync.dma_start(
                            out=xt[:], in_=x_view[t][:, :, n * AC:(n + 1) * AC]
                        )
                        x_tiles[(t, n)] = xt

                    def chain_qk(t, n, p):
                        sl = slice(n * AC, (n + 1) * AC)
                        xt = x_tiles[(t, n)]
                        ps = apsum.tile([128, AC], F32, tag="aps", name="aps")
                        for kc in range(KC):
                            nc.tensor.matmul(
                                ps[:, :],
                                w_sb[t][:, kc, p * 128:(p + 1) * 128],
                                xt[:, kc, :],
                                start=(kc == 0),
                                stop=(kc == KC - 1),
                            )
                        nc.vector.tensor_copy(
                            (qT if t == "q" else kT)[:, p, sl], ps[:, :]
                        )

                    def chain_v(n, mm):
                        xt = x_tiles[("v", n)]
                        j = n * (AC // 128) + mm
                        ps = apsum.tile([128, DG], F32, tag="aps", name="apsv")
                        for kc in range(KC):
                            nc.tensor.matmul(
                                ps[:, :],
                                xt[:, kc, mm * 128:(mm + 1) * 128],
                                w_sb["v"][:, kc, :],
                                start=(kc == 0),
                                stop=(kc == KC - 1),
                            )
                        nc.vector.tensor_copy(vaug[:, j, :, 0:HD], ps[:, :])

                    def chunk_fillers(n):
                        fs = [lambda t=t, n=n: dma_x(t, n) for t in ("q", "k", "v")]
                        for p in range(NPAIR):
                            fs.append(lambda p=p, n=n: chain_qk("q", n, p))
                        for p in range(NPAIR):
                            fs.append(lambda p=p, n=n: chain_qk("k", n, p))
                        for mm in range(AC // 128):
                            fs.append(lambda mm=mm, n=n: chain_v(n, mm))
                        return fs

                    # chunk 0: each dma_start costs ~680ns of serialized
                    # trigger time on the sync engine, so use few, large
                    # DMAs: two halves each for wq and xq0 (kc 0-3 / 4-7)
                    # keep the first chain steps consumable early.
                    xt0 = xpool.tile([128, KC, AC], BF16, tag="x", name="x_q0")
                    x_tiles[("q", 0)] = xt0
                    for h in range(2):
                        ksl = slice(4 * h, 4 * h + 4)
                        nc.sync.dma_start(
                            out=wq_sb[:, ksl, :], in_=wq_v[:, ksl, :]
                        )
                        nc.sync.dma_start(
                            out=xt0[:, ksl, :], in_=xq_v[:, ksl, 0:AC]
                        )
                    # HAM warmup: memset-fed throwaway matmuls (no DMA
                    # dependency, so they start ~immediately) woven between
                    # the DMA-gated kc steps of the first q chain -- keeps
                    # the PE busy from t=0 so the SHORT window trips early
                    # and real matmuls run at 2.4 GHz.  Borrows the ops
                    # (o2) PSUM bank, which is dead until stripe-0 PV.
                    warm = ops.tile([64, 128], F32, tag="o2", name="warm")

                    def warm_mms(n):
                        # alternate stationary slices so LDWEIGHTS of the
                        # next warm MM ping-pongs into the background
                        # weight buffer instead of serializing
                        for k in range(n):
                            c0 = 64 * (k % 2)
                            nc.tensor.matmul(
                                warm[:, :], warm_sb[:, c0:c0 + 64],
                                warm_sb[:, :],
                                start=True, stop=True,
                            )

                    # all warm MMs upfront: the PE is in-order, so warm MMs
                    # emitted after a DMA-gated matmul would stall with it.
                    warm_mms(26)
                    # kc-major: 4 pair-chains advance together, one psum each?
                    # psum only has 2 aps slots here, so do pairs in twos.
                    for ph in range(2):
                        pss = [
                            apsum.tile([128, AC], F32, tag="aps", name="q0ps")
                            for _ in range(2)
                        ]
                        for kc in range(KC):
                            for pi in range(2):
                                p = 2 * ph + pi
                                nc.tensor.matmul(
                                    pss[pi][:, :],
                                    wq_sb[:, kc, p * 128:(p + 1) * 128],
                                    xt0[:, kc, :],
                                    start=(kc == 0),
                                    stop=(kc == KC - 1),
                                )
                        for pi in range(2):
                            nc.vector.tensor_copy(
                                qT[:, 2 * ph + pi, 0:AC], pss[pi][:, :]
                            )
                    dma_x("k", 0)
                    nc.sync.dma_start(out=wk_sb[:], in_=wk_v[:, :, :])
                    nc.sync.dma_start(out=tri_sb[:], in_=tri[:, :])
                    nc.sync.dma_start(out=wv_sb[:], in_=wv_v[:, :, :])
                    dma_x("v", 0)
                    for p in range(NPAIR):
                        chain_qk("k", 0, p)
                    for mm in range(AC // 128):
                        chain_v(0, mm)

                    emit_interleaved(
                        make_stripe(0, SI, stps, ops), chunk_fillers(1)
                    )
                    emit_interleaved(
                        make_stripe(SI, SI, stps, ops), chunk_fillers(2)
                    )
                    emit_interleaved(
                        make_stripe(2 * SI, SI, stps, ops), chunk_fillers(3)
                    )

                # ---- last stripe interleaves with normalise + Wo (1-buf
                # ---- psum pools; stalls absorb into exp waits)
                with (
                    tc.tile_pool(name="st2p", bufs=2, space=MemorySpace.PSUM) as stps2,
                    tc.tile_pool(name="ops2", bufs=1, space=MemorySpace.PSUM) as ops2,
                    tc.tile_pool(name="cps1", bufs=1, space=MemorySpace.PSUM) as cps1,
                    tc.tile_pool(name="bps1", bufs=1, space=MemorySpace.PSUM) as bps1,
                ):
                    # dependency-free warm MMs bridge the pool-transition
                    # stall here: if the PE idles >one HAM window at this
                    # boundary, the clock halves and takes ~17us to recover
                    wps = bps1.tile([64, 128], F32, tag="bc", name="wps")
                    for k in range(16):
                        c0 = 64 * (k % 2)
                        nc.tensor.matmul(
                            wps[:, :], warm_sb[:, c0:c0 + 64], warm_sb[:, :],
                            start=True, stop=True,
                        )
                    c_fillers = [
                        lambda: nc.sync.dma_start(out=wo_bf[:], in_=wo_v[:, :, :])
                    ]
                    for i in range(3):
                        for p in range(NPAIR):
                            c_fillers.append(
                                lambda i=i, p=p: norm_unit(i * SI, SI, p, bps1)
                            )
                        for mt in range(8):
                            c_fillers.append(
                                lambda i=i, mt=mt: wo_chain(i * SI, SI, mt, cps1)
                            )
                    emit_interleaved(
                        make_stripe(
                            3 * SI, SI, stps2, ops2,
                            pair_done=lambda p: norm_unit(3 * SI, SI, p, bps1),
                        ),
                        c_fillers,
                        tail_frac=0.9,
                    )

            # tail: final stripe Wo, staged into dead at[] columns
            with (
                tc.tile_pool(name="cps2", bufs=4, space=MemorySpace.PSUM) as cps2,
            ):
                for mt in range(8):
                    wo_chain(3 * SI, SI, mt, cps2, stage=True)

    nc.compile()
    return nc


_NC_CACHE = {}


def _get_nc():
    if "nc" not in _NC_CACHE:
        _NC_CACHE["nc"] = _build_core_kernel()
    return _NC_CACHE["nc"]


def make_in_maps(query, key, value, Wq, Wk, Wv, Wo):
    import ml_dtypes

    bf = ml_dtypes.bfloat16
    tri = (np.arange(128)[None, :] >= np.arange(128)[:, None]).astype(bf)
    # each batch's transposed activations are shared by its two cores;
    # build them once (the bf16 casts are the expensive part on host)
    xq_b = [np.ascontiguousarray(query[b].T).astype(bf) for b in range(B)]
    xk_b = [np.ascontiguousarray(key[b].T).astype(bf) for b in range(B)]
    xv_b = [np.ascontiguousarray(value[b].T).astype(bf) for b in range(B)]
    w_g = []
    for g in range(2):
        rows = slice(g * DG, (g + 1) * DG)
        w_g.append({
            "wq": np.ascontiguousarray((Wq[rows, :] * SCALE).T).astype(bf),
            "wk": np.ascontiguousarray(Wk[rows, :].T).astype(bf),
            "wv": np.ascontiguousarray(Wv[rows, :].T).astype(bf),
            "wo": np.ascontiguousarray(Wo[:, rows].T).astype(bf),
        })
    in_maps = []
    for c in range(NCORES):
        b, g = c // 2, c % 2
        in_maps.append({
            "xq": xq_b[b], "xk": xk_b[b], "xv": xv_b[b],
            **w_g[g], "tri": tri,
        })
    return in_maps


def kernel(query, key, value, attn_mask, Wq, Wk, Wv, Wo):
    global LAST_RESULTS
    from concourse.bass_utils import run_bass_kernel_spmd

    query = np.asarray(query, np.float32)
    key = np.asarray(key, np.float32)
    value = np.asarray(value, np.float32)
    Wq = np.asarray(Wq, np.float32)
    Wk = np.asarray(Wk, np.float32)
    Wv = np.asarray(Wv, np.float32)
    Wo = np.asarray(Wo, np.float32)

    nc = _get_nc()
    in_maps = make_in_maps(query, key, value, Wq, Wk, Wv, Wo)
    res = run_bass_kernel_spmd(
        nc,
        in_maps,
        core_ids=list(range(NCORES)),
        trace=bool(int(os.environ.get("KERNEL_TRACE", "0"))),
    )
    LAST_RESULTS = res

    full = np.empty((B, S, DIM), np.float32)
    for b in range(B):
        full[b] = (
            res.results[2 * b]["out"].astype(np.float32)
            + res.results[2 * b + 1]["out"].astype(np.float32)
        ).T
    return full



# revision 63
# speedup vs baseline: 1.0254x; 1.0102x over previous
"""Distributed causal attention kernel for 8 TRN2 NeuronCores.

Sharding: core c -> (batch b = c//2, head-group g = c%2).  Each core
computes attention for its batch over 8 of the 16 heads plus the partial
output projection (row-parallel Wo); the host sums the two partials per
batch and transposes back.

Device layout (per core):
  inputs  xq/xk/xv : x.T            [1024, 2048] bf16
          wq/wk/wv : W_g.T          [1024, 512]  bf16  (SCALE folded into wq)
          wo       : Wo[:,g-cols].T [512, 1024]  bf16
          tri      : [128,128] lower-step mask  tri[p,f] = (f >= p)
  output  out      : partial O.T    [1024, 2048] bf16 (host sums in f32)

Pipeline: qT/kT = Wg @ x.T (transposed), v natural [seq, 512];
S.T[sj,si] = k q.T per head (K=64, both heads CONCURRENT via PE row
tiling -- partitions 0-63 / 64-127 -> row_grp h0/h64, two PSUM banks);
P = exp(S.T) on ScalarE (logits are tiny -> no max subtraction);
causal mask = matmul N-range restriction + tri mask on diagonal blocks;
PV with ones-augmented v (M=65) -> unnormalised A.T + denominator row;
denominators reciprocal'd once per pair on DVE (full-65-partition op:
custom DVE needs base partition 0), broadcast via tiny K=1 col-tiled
matmuls; O.T = WoT.T @ A.T, last stripe staged into dead at[] columns
so the tail ships with 2 DMA triggers instead of 8.

Start: memset-fed HAM warmup (no DMA dependency) + few big input DMAs
(each dma_start costs ~680ns serialized sync-engine trigger time).
"""

import os

import numpy as np

import concourse.bass as bass
import concourse.tile as tile
from concourse import bacc, mybir
from concourse.bass import MemorySpace

F32 = mybir.dt.float32
BF16 = mybir.dt.bfloat16
AF = mybir.ActivationFunctionType

B, S, DIM, H = 4, 2048, 1024, 16
HD = DIM // H          # 64
SCALE = HD ** -0.5
NCORES = 8
DG = DIM // 2          # 512 head dims per core (8 heads)
NPAIR = 4              # head pairs per core
SI = 512               # si chunk (query positions per attention tile)
NSI = S // SI          # 4
SJ = 128               # sj chunk (key positions per matmul)
AC = 512               # phase-A seq chunk
NAC = S // AC          # 4
KC = DIM // 128        # 8 contraction chunks for projections

LAST_RESULTS = None


def _build_core_kernel():
    nc = bacc.Bacc(
        "TRN2", target_bir_lowering=False, debug=False, num_devices=NCORES
    )

    xq = nc.dram_tensor("xq", [DIM, S], BF16, kind="ExternalInput").ap()
    xk = nc.dram_tensor("xk", [DIM, S], BF16, kind="ExternalInput").ap()
    xv = nc.dram_tensor("xv", [DIM, S], BF16, kind="ExternalInput").ap()
    wq = nc.dram_tensor("wq", [DIM, DG], BF16, kind="ExternalInput").ap()
    wk = nc.dram_tensor("wk", [DIM, DG], BF16, kind="ExternalInput").ap()
    wv = nc.dram_tensor("wv", [DIM, DG], BF16, kind="ExternalInput").ap()
    wo = nc.dram_tensor("wo", [DG, DIM], BF16, kind="ExternalInput").ap()
    tri = nc.dram_tensor("tri", [128, 128], BF16, kind="ExternalInput").ap()
    out = nc.dram_tensor("out", [DIM, S], BF16, kind="ExternalOutput").ap()

    # partition-tiled DRAM views
    xq_v = xq.rearrange("(kc p) s -> p kc s", p=128)   # [128, 8, 2048]
    xk_v = xk.rearrange("(kc p) s -> p kc s", p=128)
    xv_v = xv.rearrange("(kc p) s -> p kc s", p=128)
    wq_v = wq.rearrange("(kc p) m -> p kc m", p=128)   # [128, 8, 512]
    wk_v = wk.rearrange("(kc p) m -> p kc m", p=128)
    wv_v = wv.rearrange("(kc p) m -> p kc m", p=128)
    wo_v = wo.rearrange("(kt p) m -> p kt m", p=128)   # [128, 4, 1024]
    out_v = out.rearrange("(mt p) s -> p mt s", p=128)  # [128, 8, 2048]

    with tile.TileContext(nc) as tc:
        with (
            tc.tile_pool(name="persist", bufs=1) as persist,
            tc.tile_pool(name="cw", bufs=1) as cwpool,
            tc.tile_pool(name="co", bufs=4) as copool,
            tc.tile_pool(name="bcp", bufs=1) as bcpool,
        ):
            # persistent SBUF tensors
            qT = persist.tile([128, NPAIR, S], BF16)        # [64l+d, pair, si]
            kT = persist.tile([128, NPAIR, S], BF16)
            vaug = persist.tile([128, S // SJ, 8, HD + 1], BF16)  # [sj, j, h, d|1]
            at = persist.tile([128, NPAIR, S], BF16)        # unnorm A.T
            rden = persist.tile([65, NPAIR, 2, S], BF16)  # 1/den rows @ p64
            ones64 = persist.tile([65, 64], BF16)
            tri_sb = persist.tile([128, 128], BF16)
            warm_sb = persist.tile([128, 128], BF16)
            wo_bf = cwpool.tile([128, 4, DIM], BF16, tag="wo16")

            # ones column of vaug
            nc.vector.memset(vaug[:, :, :, HD], 1.0)
            nc.vector.memset(warm_sb[:], 0.125)
            nc.vector.memset(ones64[:], 1.0)

            def norm_unit(si0, siw, p, bpsum):
                ssl = slice(si0, si0 + siw)
                # broadcast both heads' 1/den rows (bf16, from end_pair)
                # via two tiny col-tiled K=1 matmuls, then one multiply
                bc = bpsum.tile([128, siw], F32, tag="bc", name="bc")
                for l in range(2):
                    nc.tensor.matmul(
                        bc[64 * l:64 * l + 64, :],
                        ones64[64:65, 0:64],
                        rden[64:65, p, l, ssl],
                        start=True,
                        stop=True,
                    )
                nc.vector.tensor_mul(at[:, p, ssl], at[:, p, ssl], bc[:, :])

            def wo_chain(si0, siw, mt, cpsum, stage=False):
                ssl = slice(si0, si0 + siw)
                ps = cpsum.tile([128, siw], F32, tag="cps", name="cps")
                for kt in range(4):
                    nc.tensor.matmul(
                        ps[:, :],
                        wo_bf[:, kt, mt * 128:(mt + 1) * 128],
                        at[:, kt, ssl],
                        start=(kt == 0),
                        stop=(kt == 3),
                    )
                if not stage:
                    osb = copool.tile([128, siw], BF16, tag="osb", name="osb")
                    nc.vector.tensor_copy(osb[:, :], ps[:, :])
                    nc.sync.dma_start(out=out_v[:, mt, ssl], in_=osb[:, :])
                    return
                # final stripe: stage into dead at[] columns (stripes 0-1
                # fully consumed) and ship 4 tiles per dma_start -- the
                # per-tile DMA triggers (~680ns serialized on the sync
                # engine) were most of the kernel tail.
                nc.vector.tensor_copy(
                    at[:, mt % 4, (mt // 4) * siw:(mt // 4 + 1) * siw],
                    ps[:, :],
                )
                if mt % 4 == 3:
                    h = mt // 4
                    nc.sync.dma_start(
                        out=out_v[:, 4 * h:4 * h + 4, ssl],
                        in_=at[:, :, h * siw:(h + 1) * siw],
                    )

            # Emission is interleaved so every engine's serial stream stays
            # busy: attention units (QK->exp->PV, ScalarE-bound) are the
            # backbone; projection chains / Wo chains (PE-bound) are woven
            # between them as fillers.
            with (
                tc.tile_pool(name="pt", bufs=8) as ptpool,
            ):

                def make_stripe(si0, siw, stp, opp, pair_done=None):
                    """Emission units for attention rows [si0, si0+siw)."""
                    ssl = slice(si0, si0 + siw)
                    njs = (si0 + siw) // SJ
                    units = []
                    for p in range(NPAIR):
                        state = {}

                        def start_pair(p=p, state=state):
                            state["o2"] = opp.tile(
                                [65, 2, siw], F32, tag="o2", name="o2"
                            )

                        def unit(j, p=p, state=state):
                            sj0 = j * SJ
                            d0 = sj0 - si0
                            r0 = max(0, d0)
                            # st always spans 2 PSUM banks: the two QK
                            # matmuls are concurrent row tiles, and row
                            # tiles must not write the same bank.
                            st2 = stp.tile([128, 2, SI], F32, tag="st", name="st")
                            pt = ptpool.tile([128, 2, siw], BF16, tag="pt", name="pt")
                            for l in range(2):
                                lsl = slice(64 * l, 64 * l + 64)
                                nc.tensor.matmul(
                                    st2[:, l, r0:siw],
                                    kT[lsl, p, sj0:sj0 + SJ],
                                    qT[lsl, p, si0 + r0:si0 + siw],
                                    start=True,
                                    stop=True,
                                )
                            nc.scalar.activation(
                                pt[:, :, r0:siw], st2[:, :, r0:siw], AF.Exp
                            )
                            if d0 >= 0:
                                for l in range(2):
                                    nc.vector.tensor_mul(
                                        pt[:, l, d0:d0 + 128],
                                        pt[:, l, d0:d0 + 128],
                                        tri_sb[:, :],
                                    )
                            for l in range(2):
                                nc.tensor.matmul(
                                    state["o2"][:, l, r0:siw],
                                    vaug[:, j, 2 * p + l, :],
                                    pt[:, l, r0:siw],
                                    start=(j == 0),
                                    stop=(j == njs - 1),
                                )

                        def end_pair(p=p, state=state, ssl=ssl):
                            o2 = state["o2"]
                            for l in range(2):
                                nc.vector.tensor_copy(
                                    at[64 * l:64 * l + 64, p, ssl], o2[0:HD, l, :]
                                )
                            # reciprocal of both heads' denominator rows;
                            # custom-DVE ops need base partition 0, so run
                            # over all 65 partitions -- rows 0-63 get junk
                            # reciprocals of A values that are never read.
                            # Then a lane-aligned cast of the one good row
                            # into the persistent bf16 rden.
                            rscr = bcpool.tile(
                                [65, 2, siw], F32, tag="rscr", name="rscr"
                            )
                            nc.vector.reciprocal_approx_fast(
                                rscr[0:65, :, :], o2[0:65, :, :]
                            )
                            nc.vector.tensor_copy(
                                rden[64:65, p, :, ssl], rscr[64:65, :, :]
                            )

                        units.append(start_pair)
                        for j in range(njs):
                            units.append(lambda j=j, u=unit: u(j))
                        units.append(end_pair)
                        if pair_done is not None:
                            units.append(lambda p=p: pair_done(p))
                    return units

                def emit_interleaved(units, fillers, tail_frac=1.0):
                    """Emit units with fillers distributed evenly between.

                    tail_frac < 1 exhausts the fillers by that fraction of
                    the unit stream, leaving the last units filler-free so
                    their completion chain gets idle engines.
                    """
                    U, F = len(units), len(fillers)
                    eff = max(1, int(U * tail_frac))
                    fi = 0
                    for k, u in enumerate(units):
                        u()
                        want = min(F, (k + 1) * F // eff)
                        while fi < want:
                            fillers[fi]()
                            fi += 1
                    while fi < F:
                        fillers[fi]()
                        fi += 1

                with (
                    tc.tile_pool(name="ax", bufs=3) as xpool,
                    tc.tile_pool(name="aw", bufs=1) as wpool,
                    tc.tile_pool(name="aps", bufs=2, space=MemorySpace.PSUM) as apsum,
                    tc.tile_pool(name="stps", bufs=2, space=MemorySpace.PSUM) as stps,
                    tc.tile_pool(name="ops", bufs=1, space=MemorySpace.PSUM) as ops,
                ):
                    wq_sb = wpool.tile([128, KC, DG], BF16, tag="wq")
                    wk_sb = wpool.tile([128, KC, DG], BF16, tag="wk")
                    wv_sb = wpool.tile([128, KC, DG], BF16, tag="wv")
                    w_sb = {"q": wq_sb, "k": wk_sb, "v": wv_sb}
                    x_view = {"q": xq_v, "k": xk_v, "v": xv_v}
                    x_tiles = {}

                    def dma_x(t, n):
                        xt = xpool.tile(
                            [128, KC, AC], BF16, tag="x", name=f"x_{t}{n}"
                        )
                        nc.sync.dma_start(
                            out=xt[:], in_=x_view[t][:, :, n * AC:(n + 1) * AC]
                        )
                        x_tiles[(t, n)] = xt

                    def chain_qk(t, n, p):
                        sl = slice(n * AC, (n + 1) * AC)
                        xt = x_tiles[(t, n)]
                        ps = apsum.tile([128, AC], F32, tag="aps", name="aps")
                        for kc in range(KC):
                            nc.tensor.matmul(
                                ps[:, :],
                                w_sb[t][:, kc, p * 128:(p + 1) * 128],
                                xt[:, kc, :],
                                start=(kc == 0),
                                stop=(kc == KC - 1),
                            )
                        nc.vector.tensor_copy(
                            (qT if t == "q" else kT)[:, p, sl], ps[:, :]
                        )

                    def chain_v(n, mm):
                        xt = x_tiles[("v", n)]
                        j = n * (AC // 128) + mm
                        ps = apsum.tile([128, DG], F32, tag="aps", name="apsv")
                        for kc in range(KC):
                            nc.tensor.matmul(
                                ps[:, :],
                                xt[:, kc, mm * 128:(mm + 1) * 128],
                                w_sb["v"][:, kc, :],
                                start=(kc == 0),
                                stop=(kc == KC - 1),
                            )
                        nc.vector.tensor_copy(vaug[:, j, :, 0:HD], ps[:, :])

                    def chunk_fillers(n):
                        fs = [lambda t=t, n=n: dma_x(t, n) for t in ("q", "k", "v")]
                        for p in range(NPAIR):
                            fs.append(lambda p=p, n=n: chain_qk("q", n, p))
                        for p in range(NPAIR):
                            fs.append(lambda p=p, n=n: chain_qk("k", n, p))
                        for mm in range(AC // 128):
                            fs.append(lambda mm=mm, n=n: chain_v(n, mm))
                        return fs

                    # chunk 0: each dma_start costs ~680ns of serialized
                    # trigger time on the sync engine, so use few, large
                    # DMAs: two halves each for wq and xq0 (kc 0-3 / 4-7)
                    # keep the first chain steps consumable early.
                    xt0 = xpool.tile([128, KC, AC], BF16, tag="x", name="x_q0")
                    x_tiles[("q", 0)] = xt0
                    for h in range(2):
                        ksl = slice(4 * h, 4 * h + 4)
                        nc.sync.dma_start(
                            out=wq_sb[:, ksl, :], in_=wq_v[:, ksl, :]
                        )
                        nc.sync.dma_start(
                            out=xt0[:, ksl, :], in_=xq_v[:, ksl, 0:AC]
                        )
                    # HAM warmup: memset-fed throwaway matmuls (no DMA
                    # dependency, so they start ~immediately) woven between
                    # the DMA-gated kc steps of the first q chain -- keeps
                    # the PE busy from t=0 so the SHORT window trips early
                    # and real matmuls run at 2.4 GHz.  Borrows the ops
                    # (o2) PSUM bank, which is dead until stripe-0 PV.
                    warm = ops.tile([64, 128], F32, tag="o2", name="warm")

                    def warm_mms(n):
                        # alternate stationary slices so LDWEIGHTS of the
                        # next warm MM ping-pongs into the background
                        # weight buffer instead of serializing
                        for k in range(n):
                            c0 = 64 * (k % 2)
                            nc.tensor.matmul(
                                warm[:, :], warm_sb[:, c0:c0 + 64],
                                warm_sb[:, :],
                                start=True, stop=True,
                            )

                    # all warm MMs upfront: the PE is in-order, so warm MMs
                    # emitted after a DMA-gated matmul would stall with it.
                    warm_mms(26)
                    # kc-major: 4 pair-chains advance together, one psum each?
                    # psum only has 2 aps slots here, so do pairs in twos.
                    for ph in range(2):
                        pss = [
                            apsum.tile([128, AC], F32, tag="aps", name="q0ps")
                            for _ in range(2)
                        ]
                        for kc in range(KC):
                            for pi in range(2):
                                p = 2 * ph + pi
                                nc.tensor.matmul(
                                    pss[pi][:, :],
                                    wq_sb[:, kc, p * 128:(p + 1) * 128],
                                    xt0[:, kc, :],
                                    start=(kc == 0),
                                    stop=(kc == KC - 1),
                                )
                        for pi in range(2):
                            nc.vector.tensor_copy(
                                qT[:, 2 * ph + pi, 0:AC], pss[pi][:, :]
                            )
                    dma_x("k", 0)
                    nc.sync.dma_start(out=wk_sb[:], in_=wk_v[:, :, :])
                    nc.sync.dma_start(out=tri_sb[:], in_=tri[:, :])
                    nc.sync.dma_start(out=wv_sb[:], in_=wv_v[:, :, :])
                    dma_x("v", 0)
                    # warm MMs bridge the DMA-starved window before the
                    # k chains (which gate on the full wk/xk0 transfers);
                    # without the fill HAM re-throttles here for ~7us.
                    # All fill goes BEFORE the first chain: the PE is
                    # in-order, so fill emitted after a stalled matmul
                    # would stall with it.
                    warm_mms(20)
                    for p in range(NPAIR):
                        chain_qk("k", 0, p)
                    for mm in range(AC // 128):
                        chain_v(0, mm)

                    emit_interleaved(
                        make_stripe(0, SI, stps, ops), chunk_fillers(1)
                    )
                    emit_interleaved(
                        make_stripe(SI, SI, stps, ops), chunk_fillers(2)
                    )
                    emit_interleaved(
                        make_stripe(2 * SI, SI, stps, ops), chunk_fillers(3)
                    )

                # ---- last stripe interleaves with normalise + Wo (1-buf
                # ---- psum pools; stalls absorb into exp waits)
                with (
                    tc.tile_pool(name="st2p", bufs=2, space=MemorySpace.PSUM) as stps2,
                    tc.tile_pool(name="ops2", bufs=1, space=MemorySpace.PSUM) as ops2,
                    tc.tile_pool(name="cps1", bufs=1, space=MemorySpace.PSUM) as cps1,
                    tc.tile_pool(name="bps1", bufs=1, space=MemorySpace.PSUM) as bps1,
                ):
                    # dependency-free warm MMs bridge the pool-transition
                    # stall here: if the PE idles >one HAM window at this
                    # boundary, the clock halves and takes ~17us to recover
                    wps = bps1.tile([64, 128], F32, tag="bc", name="wps")
                    for k in range(16):
                        c0 = 64 * (k % 2)
                        nc.tensor.matmul(
                            wps[:, :], warm_sb[:, c0:c0 + 64], warm_sb[:, :],
                            start=True, stop=True,
                        )
                    c_fillers = [
                        lambda: nc.sync.dma_start(out=wo_bf[:], in_=wo_v[:, :, :])
                    ]
                    for i in range(3):
                        for p in range(NPAIR):
                            c_fillers.append(
                                lambda i=i, p=p: norm_unit(i * SI, SI, p, bps1)
                            )
                        for mt in range(8):
                            c_fillers.append(
                                lambda i=i, mt=mt: wo_chain(i * SI, SI, mt, cps1)
                            )
                    emit_interleaved(
                        make_stripe(
                            3 * SI, SI, stps2, ops2,
                            pair_done=lambda p: norm_unit(3 * SI, SI, p, bps1),
                        ),
                        c_fillers,
                        tail_frac=0.9,
                    )

            # tail: final stripe Wo, staged into dead at[] columns
            with (
                tc.tile_pool(name="cps2", bufs=4, space=MemorySpace.PSUM) as cps2,
            ):
                for mt in range(8):
                    wo_chain(3 * SI, SI, mt, cps2, stage=True)

    nc.compile()
    return nc


_NC_CACHE = {}


def _get_nc():
    if "nc" not in _NC_CACHE:
        _NC_CACHE["nc"] = _build_core_kernel()
    return _NC_CACHE["nc"]


def make_in_maps(query, key, value, Wq, Wk, Wv, Wo):
    import ml_dtypes

    bf = ml_dtypes.bfloat16
    tri = (np.arange(128)[None, :] >= np.arange(128)[:, None]).astype(bf)
    # each batch's transposed activations are shared by its two cores;
    # build them once (the bf16 casts are the expensive part on host)
    xq_b = [np.ascontiguousarray(query[b].T).astype(bf) for b in range(B)]
    xk_b = [np.ascontiguousarray(key[b].T).astype(bf) for b in range(B)]
    xv_b = [np.ascontiguousarray(value[b].T).astype(bf) for b in range(B)]
    w_g = []
    for g in range(2):
        rows = slice(g * DG, (g + 1) * DG)
        w_g.append({
            "wq": np.ascontiguousarray((Wq[rows, :] * SCALE).T).astype(bf),
            "wk": np.ascontiguousarray(Wk[rows, :].T).astype(bf),
            "wv": np.ascontiguousarray(Wv[rows, :].T).astype(bf),
            "wo": np.ascontiguousarray(Wo[:, rows].T).astype(bf),
        })
    in_maps = []
    for c in range(NCORES):
        b, g = c // 2, c % 2
        in_maps.append({
            "xq": xq_b[b], "xk": xk_b[b], "xv": xv_b[b],
            **w_g[g], "tri": tri,
        })
    return in_maps


def kernel(query, key, value, attn_mask, Wq, Wk, Wv, Wo):
    global LAST_RESULTS
    from concourse.bass_utils import run_bass_kernel_spmd

    query = np.asarray(query, np.float32)
    key = np.asarray(key, np.float32)
    value = np.asarray(value, np.float32)
    Wq = np.asarray(Wq, np.float32)
    Wk = np.asarray(Wk, np.float32)
    Wv = np.asarray(Wv, np.float32)
    Wo = np.asarray(Wo, np.float32)

    nc = _get_nc()
    in_maps = make_in_maps(query, key, value, Wq, Wk, Wv, Wo)
    res = run_bass_kernel_spmd(
        nc,
        in_maps,
        core_ids=list(range(NCORES)),
        trace=bool(int(os.environ.get("KERNEL_TRACE", "0"))),
    )
    LAST_RESULTS = res

    full = np.empty((B, S, DIM), np.float32)
    for b in range(B):
        full[b] = (
            res.results[2 * b]["out"].astype(np.float32)
            + res.results[2 * b + 1]["out"].astype(np.float32)
        ).T
    return full



# revision 64
# speedup vs baseline: 1.0287x; 1.0031x over previous
"""Distributed causal attention kernel for 8 TRN2 NeuronCores.

Sharding: core c -> (batch b = c//2, head-group g = c%2).  Each core
computes attention for its batch over 8 of the 16 heads plus the partial
output projection (row-parallel Wo); the host sums the two partials per
batch and transposes back.

Device layout (per core):
  inputs  xq/xk/xv : x.T            [1024, 2048] bf16
          wq/wk/wv : W_g.T          [1024, 512]  bf16  (SCALE folded into wq)
          wo       : Wo[:,g-cols].T [512, 1024]  bf16
          tri      : [128,128] lower-step mask  tri[p,f] = (f >= p)
  output  out      : partial O.T    [1024, 2048] bf16 (host sums in f32)

Pipeline: qT/kT = Wg @ x.T (transposed), v natural [seq, 512];
S.T[sj,si] = k q.T per head (K=64, both heads CONCURRENT via PE row
tiling -- partitions 0-63 / 64-127 -> row_grp h0/h64, two PSUM banks);
P = exp(S.T) on ScalarE (logits are tiny -> no max subtraction);
causal mask = matmul N-range restriction + tri mask on diagonal blocks;
PV with ones-augmented v (M=65) -> unnormalised A.T + denominator row;
denominators reciprocal'd once per pair on DVE (full-65-partition op:
custom DVE needs base partition 0), broadcast via tiny K=1 col-tiled
matmuls; O.T = WoT.T @ A.T, last stripe staged into dead at[] columns
so the tail ships with 2 DMA triggers instead of 8.

Start: memset-fed HAM warmup (no DMA dependency) + few big input DMAs
(each dma_start costs ~680ns serialized sync-engine trigger time).
"""

import os

import numpy as np

import concourse.bass as bass
import concourse.tile as tile
from concourse import bacc, mybir
from concourse.bass import MemorySpace

F32 = mybir.dt.float32
BF16 = mybir.dt.bfloat16
AF = mybir.ActivationFunctionType

B, S, DIM, H = 4, 2048, 1024, 16
HD = DIM // H          # 64
SCALE = HD ** -0.5
NCORES = 8
DG = DIM // 2          # 512 head dims per core (8 heads)
NPAIR = 4              # head pairs per core
SI = 512               # si chunk (query positions per attention tile)
NSI = S // SI          # 4
SJ = 128               # sj chunk (key positions per matmul)
AC = 512               # phase-A seq chunk
NAC = S // AC          # 4
KC = DIM // 128        # 8 contraction chunks for projections

LAST_RESULTS = None


def _build_core_kernel():
    nc = bacc.Bacc(
        "TRN2", target_bir_lowering=False, debug=False, num_devices=NCORES
    )

    xq = nc.dram_tensor("xq", [DIM, S], BF16, kind="ExternalInput").ap()
    xk = nc.dram_tensor("xk", [DIM, S], BF16, kind="ExternalInput").ap()
    xv = nc.dram_tensor("xv", [DIM, S], BF16, kind="ExternalInput").ap()
    wq = nc.dram_tensor("wq", [DIM, DG], BF16, kind="ExternalInput").ap()
    wk = nc.dram_tensor("wk", [DIM, DG], BF16, kind="ExternalInput").ap()
    wv = nc.dram_tensor("wv", [DIM, DG], BF16, kind="ExternalInput").ap()
    wo = nc.dram_tensor("wo", [DG, DIM], BF16, kind="ExternalInput").ap()
    tri = nc.dram_tensor("tri", [128, 128], BF16, kind="ExternalInput").ap()
    out = nc.dram_tensor("out", [DIM, S], BF16, kind="ExternalOutput").ap()

    # partition-tiled DRAM views
    xq_v = xq.rearrange("(kc p) s -> p kc s", p=128)   # [128, 8, 2048]
    xk_v = xk.rearrange("(kc p) s -> p kc s", p=128)
    xv_v = xv.rearrange("(kc p) s -> p kc s", p=128)
    wq_v = wq.rearrange("(kc p) m -> p kc m", p=128)   # [128, 8, 512]
    wk_v = wk.rearrange("(kc p) m -> p kc m", p=128)
    wv_v = wv.rearrange("(kc p) m -> p kc m", p=128)
    wo_v = wo.rearrange("(kt p) m -> p kt m", p=128)   # [128, 4, 1024]
    out_v = out.rearrange("(mt p) s -> p mt s", p=128)  # [128, 8, 2048]

    with tile.TileContext(nc) as tc:
        with (
            tc.tile_pool(name="persist", bufs=1) as persist,
            tc.tile_pool(name="cw", bufs=1) as cwpool,
            tc.tile_pool(name="co", bufs=4) as copool,
            tc.tile_pool(name="bcp", bufs=1) as bcpool,
        ):
            # persistent SBUF tensors
            qT = persist.tile([128, NPAIR, S], BF16)        # [64l+d, pair, si]
            kT = persist.tile([128, NPAIR, S], BF16)
            vaug = persist.tile([128, S // SJ, 8, HD + 1], BF16)  # [sj, j, h, d|1]
            at = persist.tile([128, NPAIR, S], BF16)        # unnorm A.T
            rden = persist.tile([65, NPAIR, 2, S], BF16)  # 1/den rows @ p64
            ones64 = persist.tile([65, 64], BF16)
            tri_sb = persist.tile([128, 128], BF16)
            warm_sb = persist.tile([128, 128], BF16)
            wo_bf = cwpool.tile([128, 4, DIM], BF16, tag="wo16")

            # ones column of vaug
            nc.vector.memset(vaug[:, :, :, HD], 1.0)
            nc.vector.memset(warm_sb[:], 0.125)
            nc.vector.memset(ones64[:], 1.0)

            def norm_unit(si0, siw, p, bpsum):
                ssl = slice(si0, si0 + siw)
                # broadcast both heads' 1/den rows (bf16, from end_pair)
                # via two tiny col-tiled K=1 matmuls, then one multiply
                bc = bpsum.tile([128, siw], F32, tag="bc", name="bc")
                for l in range(2):
                    nc.tensor.matmul(
                        bc[64 * l:64 * l + 64, :],
                        ones64[64:65, 0:64],
                        rden[64:65, p, l, ssl],
                        start=True,
                        stop=True,
                    )
                nc.vector.tensor_mul(at[:, p, ssl], at[:, p, ssl], bc[:, :])

            def wo_chain(si0, siw, mt, cpsum, stage=False):
                ssl = slice(si0, si0 + siw)
                ps = cpsum.tile([128, siw], F32, tag="cps", name="cps")
                for kt in range(4):
                    nc.tensor.matmul(
                        ps[:, :],
                        wo_bf[:, kt, mt * 128:(mt + 1) * 128],
                        at[:, kt, ssl],
                        start=(kt == 0),
                        stop=(kt == 3),
                    )
                if not stage:
                    osb = copool.tile([128, siw], BF16, tag="osb", name="osb")
                    nc.vector.tensor_copy(osb[:, :], ps[:, :])
                    nc.sync.dma_start(out=out_v[:, mt, ssl], in_=osb[:, :])
                    return
                # final stripe: stage into dead at[] columns (stripes 0-1
                # fully consumed) and ship 4 tiles per dma_start -- the
                # per-tile DMA triggers (~680ns serialized on the sync
                # engine) were most of the kernel tail.
                nc.vector.tensor_copy(
                    at[:, mt % 4, (mt // 4) * siw:(mt // 4 + 1) * siw],
                    ps[:, :],
                )
                if mt % 4 == 3:
                    h = mt // 4
                    nc.sync.dma_start(
                        out=out_v[:, 4 * h:4 * h + 4, ssl],
                        in_=at[:, :, h * siw:(h + 1) * siw],
                    )

            # Emission is interleaved so every engine's serial stream stays
            # busy: attention units (QK->exp->PV, ScalarE-bound) are the
            # backbone; projection chains / Wo chains (PE-bound) are woven
            # between them as fillers.
            with (
                tc.tile_pool(name="pt", bufs=8) as ptpool,
            ):

                def make_stripe(si0, siw, stp, opp, pair_done=None):
                    """Emission units for attention rows [si0, si0+siw)."""
                    ssl = slice(si0, si0 + siw)
                    njs = (si0 + siw) // SJ
                    units = []
                    for p in range(NPAIR):
                        state = {}

                        def start_pair(p=p, state=state):
                            state["o2"] = opp.tile(
                                [65, 2, siw], F32, tag="o2", name="o2"
                            )

                        def unit(j, p=p, state=state):
                            sj0 = j * SJ
                            d0 = sj0 - si0
                            r0 = max(0, d0)
                            # st always spans 2 PSUM banks: the two QK
                            # matmuls are concurrent row tiles, and row
                            # tiles must not write the same bank.
                            st2 = stp.tile([128, 2, SI], F32, tag="st", name="st")
                            pt = ptpool.tile([128, 2, siw], BF16, tag="pt", name="pt")
                            for l in range(2):
                                lsl = slice(64 * l, 64 * l + 64)
                                nc.tensor.matmul(
                                    st2[:, l, r0:siw],
                                    kT[lsl, p, sj0:sj0 + SJ],
                                    qT[lsl, p, si0 + r0:si0 + siw],
                                    start=True,
                                    stop=True,
                                )
                            nc.scalar.activation(
                                pt[:, :, r0:siw], st2[:, :, r0:siw], AF.Exp
                            )
                            if d0 >= 0:
                                for l in range(2):
                                    nc.vector.tensor_mul(
                                        pt[:, l, d0:d0 + 128],
                                        pt[:, l, d0:d0 + 128],
                                        tri_sb[:, :],
                                    )
                            for l in range(2):
                                nc.tensor.matmul(
                                    state["o2"][:, l, r0:siw],
                                    vaug[:, j, 2 * p + l, :],
                                    pt[:, l, r0:siw],
                                    start=(j == 0),
                                    stop=(j == njs - 1),
                                )

                        def end_pair(p=p, state=state, ssl=ssl):
                            o2 = state["o2"]
                            for l in range(2):
                                nc.vector.tensor_copy(
                                    at[64 * l:64 * l + 64, p, ssl], o2[0:HD, l, :]
                                )
                            # reciprocal of both heads' denominator rows;
                            # custom-DVE ops need base partition 0, so run
                            # over all 65 partitions -- rows 0-63 get junk
                            # reciprocals of A values that are never read.
                            # Then a lane-aligned cast of the one good row
                            # into the persistent bf16 rden.
                            rscr = bcpool.tile(
                                [65, 2, siw], F32, tag="rscr", name="rscr"
                            )
                            nc.vector.reciprocal_approx_fast(
                                rscr[0:65, :, :], o2[0:65, :, :]
                            )
                            nc.vector.tensor_copy(
                                rden[64:65, p, :, ssl], rscr[64:65, :, :]
                            )

                        units.append(start_pair)
                        for j in range(njs):
                            units.append(lambda j=j, u=unit: u(j))
                        units.append(end_pair)
                        if pair_done is not None:
                            units.append(lambda p=p: pair_done(p))
                    return units

                def emit_interleaved(units, fillers, tail_frac=1.0):
                    """Emit units with fillers distributed evenly between.

                    tail_frac < 1 exhausts the fillers by that fraction of
                    the unit stream, leaving the last units filler-free so
                    their completion chain gets idle engines.
                    """
                    U, F = len(units), len(fillers)
                    eff = max(1, int(U * tail_frac))
                    fi = 0
                    for k, u in enumerate(units):
                        u()
                        want = min(F, (k + 1) * F // eff)
                        while fi < want:
                            fillers[fi]()
                            fi += 1
                    while fi < F:
                        fillers[fi]()
                        fi += 1

                with (
                    tc.tile_pool(name="ax", bufs=3) as xpool,
                    tc.tile_pool(name="aw", bufs=1) as wpool,
                    tc.tile_pool(name="aps", bufs=2, space=MemorySpace.PSUM) as apsum,
                    tc.tile_pool(name="stps", bufs=2, space=MemorySpace.PSUM) as stps,
                    tc.tile_pool(name="ops", bufs=1, space=MemorySpace.PSUM) as ops,
                ):
                    wq_sb = wpool.tile([128, KC, DG], BF16, tag="wq")
                    wk_sb = wpool.tile([128, KC, DG], BF16, tag="wk")
                    wv_sb = wpool.tile([128, KC, DG], BF16, tag="wv")
                    w_sb = {"q": wq_sb, "k": wk_sb, "v": wv_sb}
                    x_view = {"q": xq_v, "k": xk_v, "v": xv_v}
                    x_tiles = {}

                    def dma_x(t, n):
                        xt = xpool.tile(
                            [128, KC, AC], BF16, tag="x", name=f"x_{t}{n}"
                        )
                        nc.sync.dma_start(
                            out=xt[:], in_=x_view[t][:, :, n * AC:(n + 1) * AC]
                        )
                        x_tiles[(t, n)] = xt

                    def chain_qk(t, n, p):
                        sl = slice(n * AC, (n + 1) * AC)
                        xt = x_tiles[(t, n)]
                        ps = apsum.tile([128, AC], F32, tag="aps", name="aps")
                        for kc in range(KC):
                            nc.tensor.matmul(
                                ps[:, :],
                                w_sb[t][:, kc, p * 128:(p + 1) * 128],
                                xt[:, kc, :],
                                start=(kc == 0),
                                stop=(kc == KC - 1),
                            )
                        nc.vector.tensor_copy(
                            (qT if t == "q" else kT)[:, p, sl], ps[:, :]
                        )

                    def chain_v(n, mm):
                        xt = x_tiles[("v", n)]
                        j = n * (AC // 128) + mm
                        ps = apsum.tile([128, DG], F32, tag="aps", name="apsv")
                        for kc in range(KC):
                            nc.tensor.matmul(
                                ps[:, :],
                                xt[:, kc, mm * 128:(mm + 1) * 128],
                                w_sb["v"][:, kc, :],
                                start=(kc == 0),
                                stop=(kc == KC - 1),
                            )
                        nc.vector.tensor_copy(vaug[:, j, :, 0:HD], ps[:, :])

                    def chunk_fillers(n):
                        fs = [lambda t=t, n=n: dma_x(t, n) for t in ("q", "k", "v")]
                        for p in range(NPAIR):
                            fs.append(lambda p=p, n=n: chain_qk("q", n, p))
                        for p in range(NPAIR):
                            fs.append(lambda p=p, n=n: chain_qk("k", n, p))
                        for mm in range(AC // 128):
                            fs.append(lambda mm=mm, n=n: chain_v(n, mm))
                        return fs

                    # chunk 0: each dma_start costs ~680ns of serialized
                    # trigger time on the sync engine, so use few, large
                    # DMAs: two halves each for wq and xq0 (kc 0-3 / 4-7)
                    # keep the first chain steps consumable early.
                    xt0 = xpool.tile([128, KC, AC], BF16, tag="x", name="x_q0")
                    x_tiles[("q", 0)] = xt0
                    for h in range(2):
                        ksl = slice(4 * h, 4 * h + 4)
                        nc.sync.dma_start(
                            out=wq_sb[:, ksl, :], in_=wq_v[:, ksl, :]
                        )
                        nc.sync.dma_start(
                            out=xt0[:, ksl, :], in_=xq_v[:, ksl, 0:AC]
                        )
                    # HAM warmup: memset-fed throwaway matmuls (no DMA
                    # dependency, so they start ~immediately) woven between
                    # the DMA-gated kc steps of the first q chain -- keeps
                    # the PE busy from t=0 so the SHORT window trips early
                    # and real matmuls run at 2.4 GHz.  Borrows the ops
                    # (o2) PSUM bank, which is dead until stripe-0 PV.
                    warm = ops.tile([64, 128], F32, tag="o2", name="warm")

                    def warm_mms(n):
                        # alternate stationary slices so LDWEIGHTS of the
                        # next warm MM ping-pongs into the background
                        # weight buffer instead of serializing
                        for k in range(n):
                            c0 = 64 * (k % 2)
                            nc.tensor.matmul(
                                warm[:, :], warm_sb[:, c0:c0 + 64],
                                warm_sb[:, :],
                                start=True, stop=True,
                            )

                    # all warm MMs upfront: the PE is in-order, so warm MMs
                    # emitted after a DMA-gated matmul would stall with it.
                    warm_mms(26)
                    # kc-major: 4 pair-chains advance together, one psum each?
                    # psum only has 2 aps slots here, so do pairs in twos.
                    for ph in range(2):
                        pss = [
                            apsum.tile([128, AC], F32, tag="aps", name="q0ps")
                            for _ in range(2)
                        ]
                        for kc in range(KC):
                            for pi in range(2):
                                p = 2 * ph + pi
                                nc.tensor.matmul(
                                    pss[pi][:, :],
                                    wq_sb[:, kc, p * 128:(p + 1) * 128],
                                    xt0[:, kc, :],
                                    start=(kc == 0),
                                    stop=(kc == KC - 1),
                                )
                        for pi in range(2):
                            nc.vector.tensor_copy(
                                qT[:, 2 * ph + pi, 0:AC], pss[pi][:, :]
                            )
                    dma_x("k", 0)
                    nc.sync.dma_start(out=wk_sb[:], in_=wk_v[:, :, :])
                    nc.sync.dma_start(out=tri_sb[:], in_=tri[:, :])
                    nc.sync.dma_start(out=wv_sb[:], in_=wv_v[:, :, :])
                    dma_x("v", 0)
                    # warm MMs bridge the DMA-starved window before the
                    # k chains (which gate on the full wk/xk0 transfers);
                    # without the fill HAM re-throttles here for ~7us.
                    # All fill goes BEFORE the first chain: the PE is
                    # in-order, so fill emitted after a stalled matmul
                    # would stall with it.
                    warm_mms(20)
                    for p in range(NPAIR):
                        chain_qk("k", 0, p)
                    for mm in range(AC // 128):
                        chain_v(0, mm)

                    emit_interleaved(
                        make_stripe(0, SI, stps, ops), chunk_fillers(1)
                    )
                    emit_interleaved(
                        make_stripe(SI, SI, stps, ops), chunk_fillers(2)
                    )
                    emit_interleaved(
                        make_stripe(2 * SI, SI, stps, ops), chunk_fillers(3)
                    )

                # ---- last stripe interleaves with normalise + Wo (1-buf
                # ---- psum pools; stalls absorb into exp waits)
                with (
                    tc.tile_pool(name="st2p", bufs=2, space=MemorySpace.PSUM) as stps2,
                    tc.tile_pool(name="ops2", bufs=1, space=MemorySpace.PSUM) as ops2,
                    tc.tile_pool(name="cps1", bufs=1, space=MemorySpace.PSUM) as cps1,
                    tc.tile_pool(name="bps1", bufs=1, space=MemorySpace.PSUM) as bps1,
                ):
                    # dependency-free warm MMs bridge the pool-transition
                    # stall here: if the PE idles >one HAM window at this
                    # boundary, the clock halves and takes ~17us to recover
                    wps = bps1.tile([64, 128], F32, tag="bc", name="wps")
                    for k in range(16):
                        c0 = 64 * (k % 2)
                        nc.tensor.matmul(
                            wps[:, :], warm_sb[:, c0:c0 + 64], warm_sb[:, :],
                            start=True, stop=True,
                        )
                    c_fillers = [
                        lambda: nc.sync.dma_start(out=wo_bf[:], in_=wo_v[:, :, :])
                    ]
                    for i in range(3):
                        for p in range(NPAIR):
                            c_fillers.append(
                                lambda i=i, p=p: norm_unit(i * SI, SI, p, bps1)
                            )
                        for mt in range(8):
                            c_fillers.append(
                                lambda i=i, mt=mt: wo_chain(i * SI, SI, mt, cps1)
                            )
                    emit_interleaved(
                        make_stripe(
                            3 * SI, SI, stps2, ops2,
                            pair_done=lambda p: norm_unit(3 * SI, SI, p, bps1),
                        ),
                        c_fillers,
                        tail_frac=1.0,
                    )

            # tail: final stripe Wo, staged into dead at[] columns
            with (
                tc.tile_pool(name="cps2", bufs=4, space=MemorySpace.PSUM) as cps2,
            ):
                for mt in range(8):
                    wo_chain(3 * SI, SI, mt, cps2, stage=True)

    nc.compile()
    return nc


_NC_CACHE = {}


def _get_nc():
    if "nc" not in _NC_CACHE:
        _NC_CACHE["nc"] = _build_core_kernel()
    return _NC_CACHE["nc"]


def make_in_maps(query, key, value, Wq, Wk, Wv, Wo):
    import ml_dtypes

    bf = ml_dtypes.bfloat16
    tri = (np.arange(128)[None, :] >= np.arange(128)[:, None]).astype(bf)
    # each batch's transposed activations are shared by its two cores;
    # build them once (the bf16 casts are the expensive part on host)
    xq_b = [np.ascontiguousarray(query[b].T).astype(bf) for b in range(B)]
    xk_b = [np.ascontiguousarray(key[b].T).astype(bf) for b in range(B)]
    xv_b = [np.ascontiguousarray(value[b].T).astype(bf) for b in range(B)]
    w_g = []
    for g in range(2):
        rows = slice(g * DG, (g + 1) * DG)
        w_g.append({
            "wq": np.ascontiguousarray((Wq[rows, :] * SCALE).T).astype(bf),
            "wk": np.ascontiguousarray(Wk[rows, :].T).astype(bf),
            "wv": np.ascontiguousarray(Wv[rows, :].T).astype(bf),
            "wo": np.ascontiguousarray(Wo[:, rows].T).astype(bf),
        })
    in_maps = []
    for c in range(NCORES):
        b, g = c // 2, c % 2
        in_maps.append({
            "xq": xq_b[b], "xk": xk_b[b], "xv": xv_b[b],
            **w_g[g], "tri": tri,
        })
    return in_maps


def kernel(query, key, value, attn_mask, Wq, Wk, Wv, Wo):
    global LAST_RESULTS
    from concourse.bass_utils import run_bass_kernel_spmd

    query = np.asarray(query, np.float32)
    key = np.asarray(key, np.float32)
    value = np.asarray(value, np.float32)
    Wq = np.asarray(Wq, np.float32)
    Wk = np.asarray(Wk, np.float32)
    Wv = np.asarray(Wv, np.float32)
    Wo = np.asarray(Wo, np.float32)

    nc = _get_nc()
    in_maps = make_in_maps(query, key, value, Wq, Wk, Wv, Wo)
    res = run_bass_kernel_spmd(
        nc,
        in_maps,
        core_ids=list(range(NCORES)),
        trace=bool(int(os.environ.get("KERNEL_TRACE", "0"))),
    )
    LAST_RESULTS = res

    full = np.empty((B, S, DIM), np.float32)
    for b in range(B):
        full[b] = (
            res.results[2 * b]["out"].astype(np.float32)
            + res.results[2 * b + 1]["out"].astype(np.float32)
        ).T
    return full

